# revision 9
# baseline (speedup 1.0000x reference)
"""8-core TRN2 Bass kernel for the 6-layer GCN edge classifier — v3.

Changes vs v2:
- Table rows [6528/core, 256B bf16] hold TWO disjoint nodes: [hw'(r)*dinv(r) |
  hw'(r+6528)*dinv(r+6528)]. The half-choice is a SORT KEY (messages grouped
  by src half-range), so selecting a half is a compile-time slice — no DVE
  select, and the AllGather halves to 13.4MB.
- dinv_src folded into the table; dinv_dst applied once per node at
  superblock merge (dinvT replicated [64, NPC] bf16). Self-loops are plain
  messages (table value hw'*dinv, merge multiplies dinv -> dinv^2). No
  per-message norms at all.
- S one-hot tiles built in transposed [128, dst, chunk] layout so the DVE
  is_equal runs in 2x mode; matmul rhs reads S_T[:, :, j] strided.
- Chunk-major table layout + 2 pipelined AllGathers (window == chunk), so
  window-0 gathers overlap the second AllGather.
"""
import sys
import os

for _p in ("/opt/trn_rl_repo", "/root/.axon_site/_ro/trn_rl_repo"):
    if os.path.isdir(_p) and _p not in sys.path:
        sys.path.insert(0, _p)

import numpy as np
import ml_dtypes
import concourse.bass as bass
import concourse.mybir as mybir
import concourse.tile as tile
from concourse import bacc
from concourse.bass_utils import run_bass_kernel_spmd
from concourse.masks import make_identity

# problem constants
N = 100000
E = 1600000
E_OUT = 400000
IN_DIM = 16
HID = 64
OUT_DIM = 2
L = 6
BN_EPS = 1e-5

NCORES = 8
NPC_REAL = 12500
COLS = 101
NPC = 128 * COLS            # 12928 local slots
HROWS = 6528                # bounce rows per core (= 51*128); halves at +6528
NAG = 2                     # AllGather chunks (== gather windows)
CROWS = HROWS // NAG        # 3264 rows per core per chunk
WINSZ = NCORES * CROWS      # 26112 (< 32768)
TBL2 = NAG * WINSZ          # 52224
CHUNK = 2048
MAXCC = CHUNK // 128
NSB = 7
SB_COLS = [15, 15, 15, 14, 14, 14, 14]
SB_START = [0, 15, 30, 45, 59, 73, 87]
YROWS = NCORES * (NPC // 32)   # 3232 head table rows
EPC = E_OUT // NCORES

f32 = mybir.dt.float32
bf16 = mybir.dt.bfloat16
i16 = mybir.dt.int16
bf = ml_dtypes.bfloat16

_CACHE = {}


def _ceil(a, b):
    return -(-a // b)


def _wrap16(idx):
    n = idx.shape[0]
    w = idx.reshape(n // 16, 16).T.astype(np.int16)
    return np.tile(w, (8, 1))


def _sb_of(cc):
    return np.searchsorted(np.array(SB_START[1:] + [COLS]), cc, side="right")


def _preprocess(edge_index, edge_index_out, chunk=CHUNK):
    src = np.asarray(edge_index[0], dtype=np.int64)
    dst = np.asarray(edge_index[1], dtype=np.int64)
    deg = np.bincount(dst, minlength=N).astype(np.float64) + 1.0
    dinv64 = 1.0 / np.sqrt(deg)
    dinv = dinv64.astype(np.float32)

    # per-core messages (incl self loops), sorted by (sb, w, cc, half, idx)
    per_core = []
    core_of = dst // NPC_REAL
    for c in range(NCORES):
        m = core_of == c
        s_c = src[m]
        d_l = dst[m] - c * NPC_REAL
        s_all = s_c
        d_all = d_l
        sc = s_all // NPC_REAL
        sl = s_all - sc * NPC_REAL
        half = sl // HROWS               # 0 or 1 (sl < 12928 < 2*6528)
        lrow = sl - half * HROWS         # row within core slice
        j = lrow // CROWS                # AllGather chunk == window
        idx = sc * CROWS + (lrow - j * CROWS)   # window-rebased gather idx
        w = j
        cc = d_all // 128
        p = (d_all % 128).astype(np.int16)
        sb = _sb_of(cc)
        order = np.lexsort((idx, half, cc, w, sb))
        per_core.append((idx[order], half[order], p[order],
                         cc[order], w[order], sb[order]))

    # uniform block plan: (sb, w, cc, half) -> Lpad
    counts = np.zeros((NCORES, NSB, NAG, COLS, 2), np.int64)
    for c in range(NCORES):
        _, half, _, cc, w, sb = per_core[c]
        np.add.at(counts[c], (sb, w, cc, half), 1)
    blocks = []          # (sb, w, cc, half, Lpad) in stream order
    for sb in range(NSB):
        for w in range(NAG):
            for cc in range(SB_START[sb], SB_START[sb] + SB_COLS[sb]):
                got = 0
                for hf in range(2):
                    Lmax = int(counts[:, sb, w, cc, hf].max())
                    if Lmax == 0:
                        continue
                    blocks.append((sb, w, cc, hf, _ceil(Lmax, 128) * 128))
                    got += 1
                if got == 0:
                    # all-phantom column: one zero block keeps the psum
                    # accumulation chain valid (S rows all-255 -> zeros)
                    blocks.append((sb, w, cc, 0, 128))
    MTOT = sum(b[4] for b in blocks)
    MCHUNKS = MTOT // 128

    chunk_col = []
    chunk_start = []
    chunk_stop = []
    chunk_half = []
    # start/stop per (sb, w, cc) column accumulation chain (may span halves)
    key_chunks = {}
    for bi, (sb, w, cc, hf, Lp) in enumerate(blocks):
        nch = Lp // 128
        key_chunks.setdefault((sb, w, cc), []).append((bi, nch))
    bi_first = {}
    bi_last = {}
    for key, lst in key_chunks.items():
        bi_first[lst[0][0]] = True
        bi_last[lst[-1][0]] = True
    for bi, (sb, w, cc, hf, Lp) in enumerate(blocks):
        nch = Lp // 128
        col = cc - SB_START[sb]
        for j in range(nch):
            chunk_col.append(col)
            chunk_half.append(hf)
            chunk_start.append(j == 0 and bi in bi_first)
            chunk_stop.append(j == nch - 1 and bi in bi_last)

    # calls: split each (sb, w) segment into <=CHUNK pieces
    calls = []            # [w, size, chunk_base, sb, seg_end]
    cb = 0
    bi = 0
    for sb in range(NSB):
        for w in range(NAG):
            seg = 0
            while bi < len(blocks) and blocks[bi][0] == sb and blocks[bi][1] == w:
                seg += blocks[bi][4]
                bi += 1
            off = 0
            while off < seg:
                size = min(chunk, seg - off)
                calls.append([w, size, cb + off // 128, sb, False])
                off += size
            calls[-1][4] = True
            cb += seg // 128
    assert cb == MCHUNKS

    # pack per-core gather idx + slot streams
    gidx_np = []
    slot_np = []
    for c in range(NCORES):
        idx, half, p, cc, w, sb = per_core[c]
        g = np.zeros(MTOT, np.int64)
        st = np.full(MTOT, 255, np.int16)
        pos = 0
        ptr = 0
        for sbb, wb, ccb, hfb, Lp in blocks:
            nreal = int(counts[c, sbb, wb, ccb, hfb])
            sl_ = slice(ptr, ptr + nreal)
            g[pos:pos + nreal] = idx[sl_]
            st[pos:pos + nreal] = p[sl_]
            ptr += nreal
            pos += Lp
        assert ptr == len(idx) and pos == MTOT
        assert g.min() >= 0 and g.max() < WINSZ
        gidx_np.append(_wrap16(g))
        slot_np.append(np.ascontiguousarray(st.reshape(MCHUNKS, 128).T))

    # head plan (identical to v2)
    FTOT = _ceil(2 * EPC, 128) * 128
    fcalls = []
    off = 0
    while off < FTOT:
        size = min(chunk, FTOT - off)
        fcalls.append(size)
        off += size
    fidx_np = []
    fslot_np = []
    fmaps = []
    for c in range(NCORES):
        es = np.asarray(edge_index_out[0][c * EPC:(c + 1) * EPC], np.int64)
        ed = np.asarray(edge_index_out[1][c * EPC:(c + 1) * EPC], np.int64)
        gg = np.concatenate([es, ed])
        sc = gg // NPC_REAL
        sl = gg - sc * NPC_REAL
        grow = sc * (NPC // 32) + sl // 32
        gslot = (sl % 32).astype(np.int16)
        half = np.concatenate([np.zeros(EPC, np.int64), np.ones(EPC, np.int64)])
        eid = np.concatenate([np.arange(EPC), np.arange(EPC)])
        npad = FTOT - 2 * EPC
        grow = np.concatenate([grow, np.zeros(npad, np.int64)])
        gslot = np.concatenate([gslot, np.full(npad, 63, np.int16)])
        assert grow.max() < YROWS
        fidx_np.append(_wrap16(grow))
        fslot_np.append(np.ascontiguousarray(
            gslot.reshape(FTOT // 128, 128).T))
        fmaps.append((eid, half))

    plan = dict(blocks=blocks, MTOT=MTOT, MCHUNKS=MCHUNKS,
                chunk_col=chunk_col, chunk_start=chunk_start,
                chunk_stop=chunk_stop, chunk_half=chunk_half, calls=calls,
                FTOT=FTOT, fcalls=fcalls)
    return (dinv, plan, gidx_np, slot_np, fidx_np, fslot_np, fmaps)


def _build_program(plan, repeat=1, body=True, probe=None,
                   msg_bufs=3, s_bufs=2, gi_bufs=3, ag_inline=0):
    """probe: None | 'nogather' (skip gather/S/matmul) | 'nocoll' (skip
    AllGathers, gathers read stale table)"""
    calls = plan["calls"]
    MCHUNKS = plan["MCHUNKS"]
    chunk_col = plan["chunk_col"]
    chunk_start = plan["chunk_start"]
    chunk_stop = plan["chunk_stop"]
    chunk_half = plan["chunk_half"]
    FTOT = plan["FTOT"]
    fcalls = plan["fcalls"]
    MAXCC2 = max(s // 128 for _, s, _, _, _ in calls)
    FMCH = FTOT // 128
    GCOL = plan["MTOT"] // 16
    FCOL = FTOT // 16

    nc = bacc.Bacc("TRN2", target_bir_lowering=False, debug=False,
                   num_devices=NCORES, num_swdge_queues=4)

    xT_in = nc.dram_tensor("xT", [IN_DIM, NPC], f32, kind="ExternalInput")
    wemb_in = nc.dram_tensor("wemb", [IN_DIM, HID], f32, kind="ExternalInput")
    bemb_in = nc.dram_tensor("bemb", [HID, 1], f32, kind="ExternalInput")
    convw_in = nc.dram_tensor("convw", [HID, L * HID], f32, kind="ExternalInput")
    dinvT_in = nc.dram_tensor("dinvT", [HID, NPC], bf16, kind="ExternalInput")
    bng_in = nc.dram_tensor("bng", [HID, L], f32, kind="ExternalInput")
    bnb_in = nc.dram_tensor("bnb", [HID, L], f32, kind="ExternalInput")
    fcw_in = nc.dram_tensor("fcw", [HID, 4], bf16, kind="ExternalInput")
    gidx_in = nc.dram_tensor("gidx", [128, GCOL], i16, kind="ExternalInput")
    slot_in = nc.dram_tensor("slot", [128, MCHUNKS], i16, kind="ExternalInput")
    fidx_in = nc.dram_tensor("fidx", [128, FCOL], i16, kind="ExternalInput")
    fslot_in = nc.dram_tensor("fslot", [128, FMCH], i16, kind="ExternalInput")
    yout = nc.dram_tensor("yout", [FTOT, 4], f32, kind="ExternalOutput")

    with tile.TileContext(nc) as tc:
        with (
            tc.tile_pool(name="const", bufs=1) as cp,
            tc.tile_pool(name="state", bufs=1) as stp,
            tc.tile_pool(name="msg", bufs=msg_bufs) as mp,
            tc.tile_pool(name="stile", bufs=s_bufs) as Sp,
            tc.tile_pool(name="idxp", bufs=2) as ixp,
            tc.tile_pool(name="gip", bufs=gi_bufs) as gip,
            tc.tile_pool(name="small", bufs=2) as wp,
            tc.tile_pool(name="psA", bufs=1, space="PSUM") as psA,
            tc.tile_pool(name="psB", bufs=2, space="PSUM") as psB,
            tc.tile_pool(name="psC", bufs=2, space="PSUM") as psC,
            tc.tile_pool(name="dram", bufs=1, space="DRAM") as dp,
        ):
            # ---- DRAM internals
            bounce2 = dp.tile([HROWS, 128], bf16)
            ybounce = dp.tile([NPC // 32, 128], bf16)
            ytable = dp.tile([YROWS, 128], bf16, addr_space="Shared")
            arb_in = dp.tile([HID, 2], f32)
            arb_out = dp.tile([HID, 2], f32)

            # ---- constants
            id128 = cp.tile([128, 128], bf16)
            make_identity(nc, id128[:])
            iota = cp.tile([128, 128], i16)
            nc.gpsimd.iota(iota[:], pattern=[[1, 128]], base=0,
                           channel_multiplier=0)
            iotaRepT = cp.tile([128, 128, MAXCC], i16)
            nc.vector.tensor_copy(
                out=iotaRepT[:],
                in_=iota[:].rearrange("p (d o) -> p d o", o=1).to_broadcast(
                    [128, 128, MAXCC]))
            wemb = cp.tile([IN_DIM, HID], f32)
            nc.sync.dma_start(out=wemb[:], in_=wemb_in[:])
            bemb = cp.tile([HID, 1], f32)
            nc.sync.dma_start(out=bemb[:], in_=bemb_in[:])
            convw = cp.tile([HID, L * HID], f32)
            nc.sync.dma_start(out=convw[:], in_=convw_in[:])
            dinvT = cp.tile([HID, NPC], bf16)
            nc.sync.dma_start(out=dinvT[:], in_=dinvT_in[:])
            bng = cp.tile([HID, L], f32)
            nc.sync.dma_start(out=bng[:], in_=bng_in[:])
            bnb = cp.tile([HID, L], f32)
            nc.sync.dma_start(out=bnb[:], in_=bnb_in[:])
            fcw = cp.tile([HID, 4], bf16)
            nc.sync.dma_start(out=fcw[:], in_=fcw_in[:])
            slot = cp.tile([128, MCHUNKS], i16)
            nc.sync.dma_start(out=slot[:], in_=slot_in[:])
            fslot = cp.tile([128, FMCH], i16)
            nc.sync.dma_start(out=fslot[:], in_=fslot_in[:])

            # ---- state
            xeT = stp.tile([HID, NPC], f32)
            aggT = stp.tile([HID, NPC], bf16)
            hT = stp.tile([HID, NPC], bf16)       # hw'*dinv staging / scratch
            Ttile = stp.tile([128, HROWS // 128, 128], bf16)
            stat_s = stp.tile([HID, NSB], f32)
            stat_q = stp.tile([HID, NSB], f32)

            _regs = {}

            def reg_of(n):
                if n not in _regs:
                    _regs[n] = nc.gpsimd.to_reg(n)
                return _regs[n]

            mm_chunks = [(k * 512, min(512, NPC - k * 512))
                         for k in range(_ceil(NPC, 512))]
            if body:
                nc.vector.memset(Ttile[:], 0.0)
                nc.vector.memset(stat_s[:], 0.0)
                nc.vector.memset(stat_q[:], 0.0)
                nc.vector.memset(aggT[:], 0.0)

            # ---- embed
            for o, n in (mm_chunks if body else []):
                xch = ixp.tile([IN_DIM, 512], f32, tag="xch")
                nc.sync.dma_start(out=xch[:, :n], in_=xT_in[:, o:o + n])
                ps = psB.tile([HID, 512], f32, tag="mm")
                nc.tensor.matmul(out=ps[:, :n], lhsT=wemb[:],
                                 rhs=xch[:, :n], start=True, stop=True)
                nc.scalar.activation(out=xeT[:, o:o + n], in_=ps[:, :n],
                                     func=mybir.ActivationFunctionType.Identity,
                                     bias=bemb[:])

            # ---- layers
            for lit, l in enumerate([li for _ in range(repeat)
                                     for li in range(L)] if body else []):
                # hw'*dinv -> hT (bf16)
                for o, n in mm_chunks:
                    ps = psB.tile([HID, 512], f32, tag="mm")
                    nc.tensor.matmul(out=ps[:, :n],
                                     lhsT=convw[:, l * HID:(l + 1) * HID],
                                     rhs=xeT[:, o:o + n], start=True, stop=True)
                    nc.vector.tensor_tensor(out=hT[:, o:o + n], in0=ps[:, :n],
                                            in1=dinvT[:, o:o + n],
                                            op=mybir.AluOpType.mult)

                # transpose hT -> Ttile: Ttile[p, b, hf*64+f] =
                # hT[f, hf*HROWS + b*128 + p]
                TRB = 8
                nblocks = HROWS // 128   # 51
                for hf in range(2):
                    for g0 in range(0, nblocks, TRB):
                        gn = min(TRB, nblocks - g0)
                        pt = psC.tile([128, TRB, 64], bf16, tag="tr")
                        wrote = 0
                        for bb in range(gn):
                            n0 = hf * HROWS + (g0 + bb) * 128
                            if n0 >= NPC:
                                break
                            nc.tensor.transpose(
                                out=pt[:, bb, :], in_=hT[:, n0:n0 + 128],
                                identity=id128[0:64, 0:64])
                            wrote += 1
                        if wrote:
                            nc.vector.tensor_copy(
                                out=Ttile[:, g0:g0 + wrote,
                                          hf * 64:(hf + 1) * 64],
                                in_=pt[:, :wrote, :])
                nc.sync.dma_start(
                    out=bounce2[:].rearrange("(b p) f -> p b f", p=128),
                    in_=Ttile[:, :, :])

                # pipelined AllGathers (chunk-major table)
                tabs = []
                ag_emitted = [False] * NAG

                def emit_ag(j):
                    if ag_emitted[j]:
                        return
                    ag_emitted[j] = True
                    if probe != "nocoll":
                        nc.gpsimd.collective_compute(
                            "AllGather", mybir.AluOpType.bypass,
                            replica_groups=[list(range(NCORES))],
                            ins=[bounce2[j * CROWS:(j + 1) * CROWS, :].opt()],
                            outs=[tabs[j][:].opt()])

                for j in range(NAG):
                    tab = dp.tile([WINSZ, 128], bf16, addr_space="Shared",
                                  tag=f"tab{lit}_{j}", name=f"tab{lit}_{j}")
                    tabs.append(tab)
                emit_ag(0)
                if not ag_inline:
                    for j in range(1, NAG):
                        emit_ag(j)

                # gathers + S_T + matmuls
                goff = 0
                cur_psa = None
                wcnt = [0] * NAG
                for k, (w, size, cbase, sb, seg_end) in enumerate(calls):
                    cc = size // 128
                    if ag_inline:
                        emit_ag(w)
                        wcnt[w] += 1
                        if w + 1 < NAG and wcnt[w] == ag_inline:
                            emit_ag(w + 1)
                    if cur_psa is None:
                        cur_psa = psA.tile([HID, max(SB_COLS), 128], f32,
                                           tag="agg")
                    psa = cur_psa
                    skip_call = (probe == "halfgather" and k % 2 == 1)
                    if probe != "nogather" and not skip_call:
                        gi = gip.tile([128, MAXCC2 * 8], i16, tag="gi")
                        nc.sync.dma_start(
                            out=gi[:, :size // 16],
                            in_=gidx_in[:, goff:goff + size // 16])
                        msg = mp.tile([128, MAXCC2, 128], bf16, tag="msg")
                        nc.gpsimd.dma_gather(
                            out_ap=msg[:, :cc, :], in_ap=tabs[w][:, :],
                            idxs_ap=gi[:, :size // 16],
                            num_idxs=size, num_idxs_reg=reg_of(size),
                            elem_size=128, single_packet=False, queue_num=k % 4)
                        if probe not in ("nomm", "halfgather"):
                            S = Sp.tile([128, 128, MAXCC2], bf16, tag="S")
                            slb = slot[:, cbase:cbase + cc].rearrange(
                                "p (o c) -> p o c", o=1).to_broadcast(
                                [128, 128, cc])
                            nc.vector.tensor_tensor(out=S[:, :, :cc], in0=slb,
                                                    in1=iotaRepT[:, :, :cc],
                                                    op=mybir.AluOpType.is_equal)
                            for j in range(cc):
                                g = cbase + j
                                hb = chunk_half[g] * 64
                                nc.tensor.matmul(out=psa[:, chunk_col[g], :],
                                                 lhsT=msg[:, j, hb:hb + 64],
                                                 rhs=S[:, :, j],
                                                 start=chunk_start[g],
                                                 stop=chunk_stop[g])
                    goff += size // 16

                    if seg_end:
                        cur_psa = None
                        ncol = SB_COLS[sb]
                        s0 = SB_START[sb] * 128
                        n1 = ncol * 128
                        pv = psa[:, :ncol, :].rearrange("p a b -> p (a b)")
                        if probe in ("nogather", "nomm", "halfgather"):
                            continue
                        if w == 0:
                            # seed with the self-loop term (hT = hw'*dinv;
                            # the final *dinvT makes it hw'*dinv^2)
                            nc.vector.tensor_copy(
                                out=aggT[:, s0:s0 + n1],
                                in_=hT[:, s0:s0 + n1])
                            nc.vector.tensor_tensor(
                                out=aggT[:, s0:s0 + n1],
                                in0=aggT[:, s0:s0 + n1], in1=pv,
                                op=mybir.AluOpType.add)
                        else:
                            nc.vector.tensor_tensor(
                                out=aggT[:, s0:s0 + n1],
                                in0=aggT[:, s0:s0 + n1], in1=pv,
                                op=mybir.AluOpType.add)
                            nc.vector.tensor_tensor(
                                out=aggT[:, s0:s0 + n1],
                                in0=aggT[:, s0:s0 + n1],
                                in1=dinvT[:, s0:s0 + n1],
                                op=mybir.AluOpType.mult)
                            nc.vector.tensor_reduce(
                                out=stat_s[:, sb:sb + 1],
                                in_=aggT[:, s0:s0 + n1],
                                axis=mybir.AxisListType.X,
                                op=mybir.AluOpType.add)
                            nc.vector.tensor_tensor(
                                out=hT[:, s0:s0 + n1],
                                in0=aggT[:, s0:s0 + n1],
                                in1=aggT[:, s0:s0 + n1],
                                op=mybir.AluOpType.mult)
                            nc.vector.tensor_reduce(
                                out=stat_q[:, sb:sb + 1],
                                in_=hT[:, s0:s0 + n1],
                                axis=mybir.AxisListType.X,
                                op=mybir.AluOpType.add)

                # BN
                st2 = wp.tile([HID, 2], f32, tag="st2")
                nc.vector.tensor_reduce(out=st2[:, 0:1], in_=stat_s[:],
                                        axis=mybir.AxisListType.X,
                                        op=mybir.AluOpType.add)
                nc.vector.tensor_reduce(out=st2[:, 1:2], in_=stat_q[:],
                                        axis=mybir.AxisListType.X,
                                        op=mybir.AluOpType.add)
                nc.sync.dma_start(out=arb_in[:], in_=st2[:])
                nc.gpsimd.collective_compute(
                    "AllReduce", mybir.AluOpType.add,
                    replica_groups=[list(range(NCORES))],
                    ins=[arb_in[:].opt()], outs=[arb_out[:].opt()])
                gs = wp.tile([HID, 2], f32, tag="gs")
                nc.sync.dma_start(out=gs[:], in_=arb_out[:])
                mv = wp.tile([HID, 2], f32, tag="mv")
                nc.vector.tensor_scalar_mul(out=mv[:], in0=gs[:],
                                            scalar1=1.0 / N)
                var = wp.tile([HID, 1], f32, tag="var")
                nc.vector.tensor_tensor(out=var[:], in0=mv[:, 0:1],
                                        in1=mv[:, 0:1],
                                        op=mybir.AluOpType.mult)
                nc.vector.tensor_tensor(out=var[:], in0=mv[:, 1:2], in1=var[:],
                                        op=mybir.AluOpType.subtract)
                nc.vector.tensor_scalar_add(out=var[:], in0=var[:],
                                            scalar1=float(BN_EPS))
                sd = wp.tile([HID, 1], f32, tag="sd")
                nc.scalar.activation(out=sd[:], in_=var[:],
                                     func=mybir.ActivationFunctionType.Sqrt)
                rs = wp.tile([HID, 1], f32, tag="rs")
                nc.vector.reciprocal(out=rs[:], in_=sd[:])
                sc_ = wp.tile([HID, 1], f32, tag="sc")
                nc.vector.tensor_tensor(out=sc_[:], in0=bng[:, l:l + 1],
                                        in1=rs[:], op=mybir.AluOpType.mult)
                tb = wp.tile([HID, 1], f32, tag="tb")
                nc.vector.tensor_tensor(out=tb[:], in0=mv[:, 0:1], in1=sc_[:],
                                        op=mybir.AluOpType.mult)
                nc.vector.tensor_tensor(out=tb[:], in0=bnb[:, l:l + 1],
                                        in1=tb[:],
                                        op=mybir.AluOpType.subtract)
                nc.scalar.activation(out=hT[:, :], in_=aggT[:, :],
                                     func=mybir.ActivationFunctionType.Relu,
                                     scale=sc_[:], bias=tb[:])
                nc.vector.tensor_tensor(out=xeT[:], in0=xeT[:], in1=hT[:],
                                        op=mybir.AluOpType.add)

            # ---- head (same as v2)
            if body:
                nc.vector.tensor_copy(out=hT[:], in_=xeT[:])
            T3 = stp.tile([128, COLS, 4], bf16)
            for o, n in (mm_chunks if body else []):
                ps = psB.tile([HID, 512], f32, tag="mm")
                nc.tensor.matmul(out=ps[0:4, :n], lhsT=fcw[:],
                                 rhs=hT[:, o:o + n], start=True, stop=True)
                ypc = ixp.tile([4, 512], bf16, tag="ypc")
                nc.scalar.activation(
                    out=ypc[:, :n], in_=ps[0:4, :n],
                    func=mybir.ActivationFunctionType.Identity)
                nb = n // 128
                pt = psC.tile([128, 8, 64], bf16, tag="tr")
                for bb in range(nb):
                    nc.tensor.transpose(out=pt[:, bb, 0:4],
                                        in_=ypc[:, bb * 128:(bb + 1) * 128],
                                        identity=id128[0:4, 0:4])
                nc.vector.tensor_copy(out=T3[:, o // 128:o // 128 + nb, :],
                                      in_=pt[:, :nb, 0:4])
            if body:
                nc.sync.dma_start(
                    out=ybounce[:].rearrange(
                        "(b p32) (k f) -> (p32 k) b f", p32=4, k=32),
                    in_=T3[:, :, :])
                nc.gpsimd.collective_compute(
                    "AllGather", mybir.AluOpType.bypass,
                    replica_groups=[list(range(NCORES))],
                    ins=[ybounce[:].opt()], outs=[ytable[:].opt()])

            foff = 0
            fchunk = 0
            soff = 0
            for k, size in enumerate(fcalls if body else []):
                cc = size // 128
                gi = gip.tile([128, MAXCC * 8], i16, tag="gi")
                nc.sync.dma_start(out=gi[:, :size // 16],
                                  in_=fidx_in[:, foff:foff + size // 16])
                msg = mp.tile([128, MAXCC, 128], bf16, tag="msg")
                nc.gpsimd.dma_gather(
                    out_ap=msg[:, :cc, :], in_ap=ytable[:, :],
                    idxs_ap=gi[:, :size // 16],
                    num_idxs=size, num_idxs_reg=reg_of(size),
                    elem_size=128, single_packet=False, queue_num=k % 4)
                oh = Sp.tile([128, MAXCC, 32], bf16, tag="oh")
                fsb = fslot[:, fchunk:fchunk + cc].rearrange(
                    "p (c o) -> p c o", o=1).to_broadcast([128, cc, 32])
                io32 = iota[:, 0:32].rearrange(
                    "p (o d) -> p o d", o=1).to_broadcast([128, cc, 32])
                nc.vector.tensor_tensor(out=oh[:, :cc, :], in0=fsb, in1=io32,
                                        op=mybir.AluOpType.is_equal)
                prod = Sp.tile([128, MAXCC, 128], bf16, tag="prod")
                ohb = oh[:, :cc, :].rearrange(
                    "p c (k o) -> p c k o", o=1).to_broadcast([128, cc, 32, 4])
                nc.vector.tensor_tensor(
                    out=prod[:, :cc, :].rearrange("p c (k f) -> p c k f", f=4),
                    in0=msg[:, :cc, :].rearrange("p c (k f) -> p c k f", f=4),
                    in1=ohb, op=mybir.AluOpType.mult)
                yo = Sp.tile([128, MAXCC, 4], f32, tag="yo")
                nc.vector.tensor_reduce(
                    out=yo[:, :cc, :].rearrange("p c (o k) -> p c o k", o=4),
                    in_=prod[:, :cc, :].rearrange("p c (k o) -> p c o k", o=4),
                    axis=mybir.AxisListType.X, op=mybir.AluOpType.add)
                nc.sync.dma_start(
                    out=yout[soff:soff + size, :].rearrange(
                        "(c p) f -> p c f", p=128),
                    in_=yo[:, :cc, :])
                foff += size // 16
                fchunk += cc
                soff += size
            if not body:
                yo0 = Sp.tile([128, MAXCC, 4], f32, tag="yo")
                nc.vector.memset(yo0[:], 0.0)
                nc.sync.dma_start(
                    out=yout[0:MAXCC * 128, :].rearrange(
                        "(c p) f -> p c f", p=128),
                    in_=yo0[:, :, :])
    nc.compile()
    return nc


def _prepare(inputs):
    edge_index = np.asarray(inputs["edge_index"])
    edge_index_out = np.asarray(inputs["edge_index_out"])
    key = hash((edge_index[0, :50].tobytes(), edge_index_out[0, :50].tobytes()))
    if key in _CACHE:
        return _CACHE[key]
    pre = _preprocess(edge_index, edge_index_out)
    nc = _build_program(pre[1], ag_inline=6)
    _CACHE[key] = (pre, nc)
    return _CACHE[key]


def _input_maps(pre, inputs):
    (dinv, plan, gidx_np, slot_np, fidx_np, fslot_np, fmaps) = pre
    x = np.asarray(inputs["x"], np.float32)
    W_emb = np.asarray(inputs["W_emb"], np.float32)
    b_emb = np.asarray(inputs["b_emb"], np.float32)
    conv_W = np.asarray(inputs["conv_W"], np.float32)
    bn_gamma = np.asarray(inputs["bn_gamma"], np.float32)
    bn_beta = np.asarray(inputs["bn_beta"], np.float32)
    fc_W = np.asarray(inputs["fc_W"], np.float32)

    fcw_cat = np.concatenate([fc_W[:HID], fc_W[HID:]], axis=1).astype(bf)
    convw_cat = np.ascontiguousarray(
        np.transpose(conv_W, (1, 0, 2)).reshape(HID, L * HID))
    in_maps = []
    for c in range(NCORES):
        xs = np.zeros((NPC, IN_DIM), np.float32)
        xs[:NPC_REAL] = x[c * NPC_REAL:(c + 1) * NPC_REAL]
        dv = np.zeros(NPC, np.float32)
        dv[:NPC_REAL] = dinv[c * NPC_REAL:(c + 1) * NPC_REAL]
        dvT = np.tile(dv[None, :], (HID, 1)).astype(bf)
        in_maps.append(dict(
            xT=np.ascontiguousarray(xs.T),
            wemb=W_emb,
            bemb=np.ascontiguousarray(b_emb[:, None]),
            convw=convw_cat,
            dinvT=np.ascontiguousarray(dvT),
            bng=np.ascontiguousarray(bn_gamma.T),
            bnb=np.ascontiguousarray(bn_beta.T),
            fcw=fcw_cat,
            gidx=gidx_np[c], slot=slot_np[c],
            fidx=fidx_np[c], fslot=fslot_np[c],
        ))
    return in_maps


def kernel(x, edge_index, edge_index_out, W_emb, b_emb, conv_W, conv_b,
           bn_gamma, bn_beta, fc_W, fc_b):
    pre, nc = _prepare(dict(edge_index=edge_index,
                            edge_index_out=edge_index_out))
    (dinv, plan, gidx_np, slot_np, fidx_np, fslot_np, fmaps) = pre
    in_maps = _input_maps(pre, dict(
        x=x, W_emb=W_emb, b_emb=b_emb, conv_W=conv_W,
        bn_gamma=bn_gamma, bn_beta=bn_beta, fc_W=fc_W))
    res = run_bass_kernel_spmd(nc, in_maps, core_ids=list(range(NCORES)))

    fc_b = np.asarray(fc_b, np.float32)
    out = np.zeros((E_OUT, OUT_DIM), np.float32)
    for c in range(NCORES):
        y = res.results[c]["yout"]          # [FTOT, 4]
        eid, half = fmaps[c]
        vals = y[:2 * EPC]
        sel = half == 0
        out[c * EPC + eid[sel], :] += vals[sel][:, 0:2]
        out[c * EPC + eid[~sel], :] += vals[~sel][:, 2:4]
    out += fc_b[None, :]
    return out
